# revision 11
# baseline (speedup 1.0000x reference)
"""Two-layer GAT (PyG GATConv semantics) on 8 Trainium2 NeuronCores.

Strategy (graph/data parallel, per sharding hint):
  - Edges (incl. self-loops) are sorted by destination and sharded by dst
    node range across the 8 cores. Each core runs the same SPMD program.
  - Per layer, each core computes the full node table
    row(n) = [h0(n) | 1 | h1(n) | 1 | al_src(n) | al_dst(n)]  (134 f32)
    via a PE matmul of x^T tiles against an augmented weight matrix
    (attention projections al = x @ (W . a) are folded into the matmul).
  - Edge phase: 128-edge chunks whose dsts lie in a 16-wide window.
    Indirect-DMA gathers fetch table rows by src (528B rows) and the
    8B al_dst pair by dst. Attention weights w = exp(leakyrelu(als+ald))
    are computed batched; a per-chunk "weighted one-hot" matrix
    S'[e, d] = (iota==dst_off) * w  (one per head, stacked [128, 32])
    turns segment-sum into a PE matmul: S'.T @ [h | 1] accumulates both
    the numerator and the softmax denominator per dst.
  - Per 4 chunks the [128, 130] PSUM tile is copied to SBUF and
    scatter-added (dma_scatter_add) into a DRAM accumulator indexed by
    local dst (head1 offset + dummy rows absorb the cross-head garbage
    halves of the combined-stationary matmul).
  - Finalize: out = numer/denom per head, then concat+bias+ELU (layer 1)
    or head-mean+bias (layer 2).
  - The layer-1 output shards are gathered on the host between the two
    launches (free for device time), so no device collectives are needed.
"""

import sys

sys.path.insert(0, "/opt/trn_rl_repo")

import math
from contextlib import ExitStack
from dataclasses import dataclass

import numpy as np

import concourse.bass as bass
import concourse.mybir as mybir
import concourse.tile as tile
from concourse.bass import IndirectOffsetOnAxis
from concourse.bass_utils import run_bass_kernel_spmd
from concourse.vector_clock import ScopedClock

F32 = mybir.dt.float32
I32 = mybir.dt.int32
I16 = mybir.dt.int16

P = 128  # partitions
WIN = 16  # dst window width per chunk
CHUNK = 128  # edges per chunk
FLUSH = 4  # chunks per psum flush group (4*32 = 128 psum rows)
GG = 32  # chunks per gather group (one indirect-DMA batch)
TBL_W = 134  # table row: h0(64) 1 h1(64) 1 als(2) ald(2)


class PatchedTC(tile.TileContext):
    """This container's walrus allows only one sync-wait on the SP CTRL
    (Drain) encoding; TileContext's kernel-tail drain attaches one wait per
    active semaphore. Split them across chained drains (SP executes in
    order, so all waits still gate the barrier)."""

    MAX_DRAIN_WAITS = 1

    def _drain_and_barrier(self, tick_clock, wait_clock):
        drain_inst = self.nc.sync.drain()
        wait_clock.add_sem_waits(
            drain_inst.ins, ScopedClock({None: tick_clock.global_clock})
        )
        si = drain_inst.ins.sync_info
        if si is not None and len(si.on_wait) > self.MAX_DRAIN_WAITS:
            waits = list(si.on_wait)
            si.on_wait = waits[: self.MAX_DRAIN_WAITS]
            rest = waits[self.MAX_DRAIN_WAITS :]
            while rest:
                d2 = self.nc.sync.drain()
                s2 = d2.ins.sync_info
                chunk, rest = rest[: self.MAX_DRAIN_WAITS], rest[self.MAX_DRAIN_WAITS :]
                if s2 is None:
                    d2.ins.sync_info = mybir.SyncInfo(on_wait=chunk, on_update=[])
                else:
                    s2.on_wait = chunk
        self.nc.all_engine_barrier()
        assert self.sems is not None
        popped = self.nc._tile_sem_poison_stack.pop()
        assert popped is self._sem_poison
        self.nc.clear_and_free_semaphores(list(self.sems.allocated().values()))
        self.nc.all_engine_barrier()


@dataclass(frozen=True)
class Cfg:
    n: int  # number of real nodes
    n_cores: int

    @property
    def nshard(self):  # real dst nodes per core
        return self.n // self.n_cores

    @property
    def nlocal(self):  # padded local dst rows (mult of 128)
        return ((self.nshard + P - 1) // P) * P

    @property
    def npad(self):  # padded global node rows (mult of 512 for xT DMA tiles)
        return ((self.n + 511) // 512) * 512

    @property
    def acc_h(self):  # accumulator rows per (head, parity) region
        return self.nlocal + 256


FULL = Cfg(n=100000, n_cores=8)


def _split_sync_waits(nc, max_waits=1):
    """This walrus build accepts at most one sync-wait command per
    instruction. Hoist extra waits onto same-engine NoOps inserted just
    before the instruction (engines execute in order, so the instruction
    is still gated by every original wait)."""
    uid = 0
    for fn in nc.m.functions:
        for bb in fn.blocks:
            new = []
            for ins in bb.instructions:
                si = ins.sync_info
                if si is not None and len(si.on_wait) > max_waits:
                    waits = list(si.on_wait)
                    for w in waits[:-max_waits]:
                        nop = mybir.InstNoOp(name=f"waitnop-{uid}", ins=[], outs=[])
                        uid += 1
                        nop.engine = ins.engine
                        nop.sync_info = mybir.SyncInfo(on_wait=[w], on_update=[])
                        nc.register_instruction(nop, overwrite=True)
                        new.append(nop)
                    si.on_wait = waits[-max_waits:]
                new.append(ins)
            bb.instructions = new


# ----------------------------------------------------------------- host prep


def prep_edges(cfg: Cfg, edge_index: np.ndarray):
    """Sort (edges + self-loops) by dst, shard by dst range, pack into
    128-edge chunks with dst confined to a 16-wide window, and emit all
    per-core device metadata arrays."""
    n, ncores = cfg.n, cfg.n_cores
    src = np.concatenate([edge_index[0], np.arange(n, dtype=np.int64)])
    dst = np.concatenate([edge_index[1], np.arange(n, dtype=np.int64)])
    order = np.argsort(dst, kind="stable")
    src = src[order].astype(np.int32)
    dst = dst[order].astype(np.int32)

    bounds = np.searchsorted(dst, np.arange(ncores + 1) * cfg.nshard)
    per_core_chunks = []  # list of list of (start, end, win0)
    for c in range(ncores):
        lo, hi = int(bounds[c]), int(bounds[c + 1])
        dloc = dst[lo:hi] - c * cfg.nshard
        chunks = []
        i = 0
        ne = hi - lo
        while i < ne:
            j_cap = min(i + CHUNK, ne)
            j_win = int(np.searchsorted(dloc, dloc[i] + WIN, side="left"))
            j = min(j_cap, j_win)
            chunks.append((lo + i, lo + j, int(dloc[i])))
            i = j
        per_core_chunks.append(chunks)

    nchunk = max(len(ch) for ch in per_core_chunks)
    nchunk = ((nchunk + GG - 1) // GG) * GG
    ng = nchunk // GG

    meta = []
    for c in range(ncores):
        chunks = per_core_chunks[c]
        src_idx = np.zeros((P, nchunk), np.int32)
        dst_idx = np.zeros((P, nchunk), np.int32)
        dst_off = np.full((P, nchunk), float(WIN), np.float32)
        # scatter idx arrays [128, ng*8] int32: value at [p, g*8+slot] =
        # accum row for staging row (p, slot) of group g. Accum regions:
        # (head0,even) (head0,odd) (head1,even) (head1,odd) x acc_h rows;
        # chunk-parity split keeps concurrent CCE-add descriptors on
        # disjoint rows (a dst straddles only adjacent chunks).
        scatA = np.zeros((P, ng * 8), np.int32)
        scatB = np.zeros((P, ng * 8), np.int32)
        win0s = np.zeros(nchunk, np.int64)
        has_chunk = np.zeros(nchunk, bool)
        for k, (a, b, w0) in enumerate(chunks):
            cnt = b - a
            src_idx[:cnt, k] = src[a:b]
            dst_idx[:cnt, k] = dst[a:b]
            dst_off[:cnt, k] = (dst[a:b] - c * cfg.nshard - w0).astype(np.float32)
            win0s[k] = w0
            has_chunk[k] = True
        dummy = cfg.nlocal + 1
        for g in range(ng):
            for slot in range(8):
                for p in range(P):
                    k = g * GG + slot * FLUSH + p // 32
                    r = p % 32
                    q = k % 2
                    base0 = q * cfg.acc_h
                    base1 = (2 + q) * cfg.acc_h
                    if has_chunk[k] and r < 16:
                        va = base0 + win0s[k] + r
                    else:
                        va = base0 + dummy
                    if has_chunk[k] and r >= 16:
                        vb = base1 + win0s[k] + (r - 16)
                    else:
                        vb = base1 + dummy
                    scatA[p, g * 8 + slot] = va
                    scatB[p, g * 8 + slot] = vb
        meta.append(
            dict(
                src_idx=src_idx,
                dst_idx=dst_idx,
                dst_off=dst_off,
                scatA=scatA,
                scatB=scatB,
            )
        )
    return nchunk, meta


def make_w_aug(W, a_src, a_dst):
    """[F_in, 134] augmented weight: W cols split around zero 'ones' slots,
    al_src/al_dst projections folded in (al = x @ (W.reshape . a))."""
    f_in = W.shape[0]
    h = a_src.shape[0]
    ch = W.shape[1] // h
    Wr = W.reshape(f_in, h, ch)
    Wa = np.einsum("fhc,hc->fh", Wr, a_src)
    Wd = np.einsum("fhc,hc->fh", Wr, a_dst)
    out = np.zeros((f_in, TBL_W), np.float32)
    out[:, 0:ch] = Wr[:, 0, :]
    out[:, ch + 1 : 2 * ch + 1] = Wr[:, 1, :]
    out[:, 130:132] = Wa
    out[:, 132:134] = Wd
    return out


# ------------------------------------------------------------ device program


def build_program(cfg: Cfg, nchunk: int, layer: int):
    """Build the SPMD bass program for one GAT layer. layer=1: out [nlocal,
    128] = ELU(concat-head GAT)+b; layer=2: out [nlocal, 64] = mean-head
    GAT+b."""
    out_w = 128 if layer == 1 else 64
    ng = nchunk // GG
    ntile = cfg.npad // P
    acc_rows = 4 * cfg.acc_h

    nc = bass.Bass(
        "TRN2", target_bir_lowering=False, debug=False, num_devices=cfg.n_cores
    )
    xT = nc.dram_tensor("xT", [P, cfg.npad], F32, kind="ExternalInput").ap()
    w_aug = nc.dram_tensor("w_aug", [P, TBL_W], F32, kind="ExternalInput").ap()
    bias = nc.dram_tensor("bias", [P, out_w], F32, kind="ExternalInput").ap()
    src_idx = nc.dram_tensor("src_idx", [P, nchunk], I32, kind="ExternalInput").ap()
    dst_idx = nc.dram_tensor("dst_idx", [P, nchunk], I32, kind="ExternalInput").ap()
    dst_off = nc.dram_tensor("dst_off", [P, nchunk], F32, kind="ExternalInput").ap()
    scatA = nc.dram_tensor("scatA", [P, ng * 8], I32, kind="ExternalInput").ap()
    scatB = nc.dram_tensor("scatB", [P, ng * 8], I32, kind="ExternalInput").ap()
    out = nc.dram_tensor("out", [cfg.nlocal, out_w], F32, kind="ExternalOutput").ap()
    table = nc.dram_tensor("table", [cfg.npad, TBL_W], F32).ap()
    accum = nc.dram_tensor("accum", [acc_rows, 65], F32).ap()

    with PatchedTC(nc) as tc, ExitStack() as ctx:
        cpool = ctx.enter_context(tc.tile_pool(name="const", bufs=1))

        # --- constants / metadata into SBUF
        src_t = cpool.tile([P, nchunk], I32)
        nc.sync.dma_start(src_t[:], src_idx[:])
        dst_t = cpool.tile([P, nchunk], I32)
        nc.sync.dma_start(dst_t[:], dst_idx[:])
        off_t = cpool.tile([P, nchunk], F32)
        nc.sync.dma_start(off_t[:], dst_off[:])
        scA_t = cpool.tile([P, ng * 8], I32)
        nc.sync.dma_start(scA_t[:], scatA[:])
        scB_t = cpool.tile([P, ng * 8], I32)
        nc.sync.dma_start(scB_t[:], scatB[:])
        wa_t = cpool.tile([P, TBL_W], F32)
        nc.sync.dma_start(wa_t[:], w_aug[:])
        bias_t = cpool.tile([P, out_w], F32)
        nc.sync.dma_start(bias_t[:], bias[:])

        iota_i = cpool.tile([P, WIN], I32)
        nc.gpsimd.iota(iota_i[:], pattern=[[1, WIN]], base=0, channel_multiplier=0)
        iota_f = cpool.tile([P, WIN], F32)
        nc.vector.tensor_copy(iota_f[:], iota_i[:])

        zero_t = cpool.tile([P, 2048], F32)
        nc.vector.memset(zero_t[:], 0.0)

        # --- phase A0: zero the accumulator
        zrows = 3072
        for r in range(0, acc_rows, zrows):
            rr = min(zrows, acc_rows - r)
            # 65 f32 per row; zero_t reinterpreted by element count only
            nc.sync.dma_start(accum[r : r + rr, :], zero_t[:, : (rr * 65) // P])

        # --- phase A: node table = xT.T @ w_aug (+ baked 1.0 columns)
        tpsum = ctx.enter_context(tc.tile_pool(name="tpsum", bufs=3, space="PSUM"))
        xpool = ctx.enter_context(tc.tile_pool(name="xt", bufs=3))
        spool_a = ctx.enter_context(tc.tile_pool(name="tstgA", bufs=1))
        spool_b = ctx.enter_context(tc.tile_pool(name="tstgB", bufs=1))
        stgs = [
            spool_a.tile([P, TBL_W], F32, name="tstgA"),
            spool_b.tile([P, TBL_W], F32, name="tstgB"),
        ]
        for s in stgs:
            nc.vector.memset(s[:, 64:65], 1.0)
            nc.vector.memset(s[:, 129:130], 1.0)
        for i in range(0, ntile, 4):
            xt = xpool.tile([P, 4 * P], F32)
            nc.sync.dma_start(xt[:], xT[:, i * P : (i + 4) * P])
            for j in range(4):
                t = i + j
                ps = tpsum.tile([P, TBL_W], F32)
                nc.tensor.matmul(
                    ps[:, :],
                    lhsT=xt[:, j * P : (j + 1) * P],
                    rhs=wa_t[:, :],
                    start=True,
                    stop=True,
                )
                stg = stgs[t % 2]
                nc.vector.tensor_copy(stg[:, 0:64], ps[:, 0:64])
                nc.vector.tensor_copy(stg[:, 65:129], ps[:, 65:129])
                nc.vector.tensor_copy(stg[:, 130:134], ps[:, 130:134])
                nc.sync.dma_start(table[t * P : (t + 1) * P, :], stg[:, :])

        tc.strict_bb_all_engine_barrier()

        # --- phase B: edge message passing
        gpool = ctx.enter_context(tc.tile_pool(name="gath", bufs=2))
        adpool = ctx.enter_context(tc.tile_pool(name="ald", bufs=2))
        wpool = ctx.enter_context(tc.tile_pool(name="wts", bufs=2))
        sppool = ctx.enter_context(tc.tile_pool(name="sprime", bufs=4))
        epsum = ctx.enter_context(tc.tile_pool(name="epsum", bufs=4, space="PSUM"))
        stApool = ctx.enter_context(tc.tile_pool(name="stA", bufs=2))
        stBpool = ctx.enter_context(tc.tile_pool(name="stB", bufs=2))

        for g in range(ng):
            gt = gpool.tile([P, GG, TBL_W], F32)
            nc.gpsimd.indirect_dma_start(
                out=gt[:],
                out_offset=None,
                in_=table[:, :],
                in_offset=IndirectOffsetOnAxis(
                    ap=src_t[:, g * GG : (g + 1) * GG], axis=0
                ),
            )
            ad = adpool.tile([P, GG, 2], F32)
            nc.gpsimd.indirect_dma_start(
                out=ad[:],
                out_offset=None,
                in_=table[:, :],
                in_offset=IndirectOffsetOnAxis(
                    ap=dst_t[:, g * GG : (g + 1) * GG], axis=0
                ),
                element_offset=132,
            )
            # w = exp(max(t, 0.2t)), t = al_src[src] + al_dst[dst]
            tsum = wpool.tile([P, GG, 2], F32, tag="tsum")
            nc.vector.tensor_add(tsum[:], gt[:, :, 130:132], ad[:])
            tscaled = wpool.tile([P, GG, 2], F32, tag="tscaled")
            nc.vector.tensor_scalar_mul(tscaled[:], tsum[:], 0.2)
            nc.vector.tensor_max(tsum[:], tsum[:], tscaled[:])
            wt = wpool.tile([P, GG, 2], F32, tag="wt")
            nc.scalar.activation(wt[:], tsum[:], mybir.ActivationFunctionType.Exp)

            stA = stApool.tile([P, 8, 65], F32)
            stB = stBpool.tile([P, 8, 65], F32)
            for jj in range(GG):
                k = g * GG + jj
                sp = sppool.tile([P, 2 * WIN], F32)
                nc.vector.tensor_scalar(
                    sp[:, 0:WIN],
                    iota_f[:],
                    off_t[:, k : k + 1],
                    wt[:, jj, 0:1],
                    op0=mybir.AluOpType.is_equal,
                    op1=mybir.AluOpType.mult,
                )
                nc.vector.tensor_scalar(
                    sp[:, WIN : 2 * WIN],
                    iota_f[:],
                    off_t[:, k : k + 1],
                    wt[:, jj, 1:2],
                    op0=mybir.AluOpType.is_equal,
                    op1=mybir.AluOpType.mult,
                )
                c = jj % FLUSH
                if c == 0:
                    pe = epsum.tile([P, 130], F32)
                nc.tensor.matmul(
                    pe[c * 32 : (c + 1) * 32, 0:65],
                    lhsT=sp[:, :],
                    rhs=gt[:, jj, 0:65],
                    start=True,
                    stop=True,
                    tile_position=(0, c * 32),
                )
                nc.tensor.matmul(
                    pe[c * 32 : (c + 1) * 32, 65:130],
                    lhsT=sp[:, :],
                    rhs=gt[:, jj, 65:130],
                    start=True,
                    stop=True,
                    tile_position=(0, c * 32),
                )
                if c == FLUSH - 1:
                    f = jj // FLUSH
                    nc.vector.tensor_copy(stA[:, f, :], pe[:, 0:65])
                    nc.vector.tensor_copy(stB[:, f, :], pe[:, 65:130])
            nc.gpsimd.indirect_dma_start(
                out=accum[:, :],
                out_offset=IndirectOffsetOnAxis(
                    ap=scA_t[:, g * 8 : (g + 1) * 8], axis=0
                ),
                in_=stA[:],
                in_offset=None,
                compute_op=mybir.AluOpType.add,
            )
            nc.gpsimd.indirect_dma_start(
                out=accum[:, :],
                out_offset=IndirectOffsetOnAxis(
                    ap=scB_t[:, g * 8 : (g + 1) * 8], axis=0
                ),
                in_=stB[:],
                in_offset=None,
                compute_op=mybir.AluOpType.add,
            )

        tc.strict_bb_all_engine_barrier()

        # --- phase C: finalize (divide by denom, bias, activation)
        fpool = ctx.enter_context(tc.tile_pool(name="fin", bufs=3))
        for t in range(cfg.nlocal // P):
            a0 = fpool.tile([P, 65], F32, tag="a0")
            nc.sync.dma_start(a0[:], accum[t * P : (t + 1) * P, :])
            a0b = fpool.tile([P, 65], F32, tag="a0b")
            nc.sync.dma_start(
                a0b[:], accum[cfg.acc_h + t * P : cfg.acc_h + (t + 1) * P, :]
            )
            a1 = fpool.tile([P, 65], F32, tag="a1")
            nc.sync.dma_start(
                a1[:],
                accum[2 * cfg.acc_h + t * P : 2 * cfg.acc_h + (t + 1) * P, :],
            )
            a1b = fpool.tile([P, 65], F32, tag="a1b")
            nc.sync.dma_start(
                a1b[:],
                accum[3 * cfg.acc_h + t * P : 3 * cfg.acc_h + (t + 1) * P, :],
            )
            nc.vector.tensor_add(a0[:], a0[:], a0b[:])
            nc.vector.tensor_add(a1[:], a1[:], a1b[:])
            r0 = fpool.tile([P, 1], F32, tag="r0")
            nc.vector.reciprocal(r0[:], a0[:, 64:65])
            r1 = fpool.tile([P, 1], F32, tag="r1")
            nc.vector.reciprocal(r1[:], a1[:, 64:65])
            if layer == 1:
                o = fpool.tile([P, 128], F32, tag="o")
                nc.vector.tensor_scalar(
                    o[:, 0:64], a0[:, 0:64], r0[:], None, op0=mybir.AluOpType.mult
                )
                nc.vector.tensor_scalar(
                    o[:, 64:128], a1[:, 0:64], r1[:], None, op0=mybir.AluOpType.mult
                )
                # o += b, then ELU(o) = relu(o) + exp(min(o,0)) - 1
                nc.vector.tensor_add(o[:], o[:], bias_t[:])
                u = fpool.tile([P, 128], F32, tag="u")
                nc.vector.tensor_scalar_min(u[:], o[:], 0.0)
                e = fpool.tile([P, 128], F32, tag="e")
                nc.scalar.activation(e[:], u[:], mybir.ActivationFunctionType.Exp)
                rl = fpool.tile([P, 128], F32, tag="rl")
                nc.scalar.activation(rl[:], o[:], mybir.ActivationFunctionType.Relu)
                nc.vector.tensor_add(e[:], e[:], rl[:])
                ot = fpool.tile([P, out_w], F32, tag="ot")
                nc.vector.tensor_scalar_add(ot[:], e[:], -1.0)
            else:
                # 0.5 * (n0/d0 + n1/d1) + b
                nc.scalar.mul(r0[:], r0[:], 0.5)
                nc.scalar.mul(r1[:], r1[:], 0.5)
                m0 = fpool.tile([P, 64], F32, tag="m0")
                nc.vector.tensor_scalar(
                    m0[:], a0[:, 0:64], r0[:], None, op0=mybir.AluOpType.mult
                )
                m1 = fpool.tile([P, 64], F32, tag="m1")
                nc.vector.tensor_scalar(
                    m1[:], a1[:, 0:64], r1[:], None, op0=mybir.AluOpType.mult
                )
                nc.vector.tensor_add(m0[:], m0[:], m1[:])
                ot = fpool.tile([P, out_w], F32, tag="ot")
                nc.vector.tensor_add(ot[:], m0[:], bias_t[:])
            nc.sync.dma_start(out[t * P : (t + 1) * P, :], ot[:])

    _split_sync_waits(nc)
    return nc


# ----------------------------------------------------------------- execution


def _pad_rows(a: np.ndarray, rows: int) -> np.ndarray:
    outp = np.zeros((rows, a.shape[1]), a.dtype)
    outp[: a.shape[0]] = a
    return outp


def run_layer(cfg: Cfg, nchunk, meta, x_full, W, a_src, a_dst, b, layer, runner=None):
    """x_full: [n, f_in] f32. Returns [n, out_w] f32 (layer output for all
    nodes, assembled from per-core dst shards)."""
    nc = build_program(cfg, nchunk, layer)
    out_w = 128 if layer == 1 else 64
    xT = np.ascontiguousarray(_pad_rows(x_full, cfg.npad).T)
    w_aug = make_w_aug(W, a_src, a_dst)
    bias_row = b.astype(np.float32)
    bias_t = np.broadcast_to(bias_row, (P, out_w)).copy()
    in_maps = []
    for c in range(cfg.n_cores):
        m = meta[c]
        in_maps.append(
            {
                "xT": xT,
                "w_aug": w_aug,
                "bias": bias_t,
                "src_idx": m["src_idx"],
                "dst_idx": m["dst_idx"],
                "dst_off": m["dst_off"],
                "scatA": m["scatA"],
                "scatB": m["scatB"],
            }
        )
    if runner is None:
        res = run_bass_kernel_spmd(nc, in_maps, list(range(cfg.n_cores)))
        outs = [res.results[c]["out"] for c in range(cfg.n_cores)]
    else:
        outs = runner(nc, in_maps)
    h = np.concatenate([o[: cfg.nshard] for o in outs], axis=0)
    return h[: cfg.n]


def kernel(x, edge_index, W1, a_src1, a_dst1, b1, W2, a_src2, a_dst2, b2):
    cfg = FULL
    x = np.asarray(x, np.float32)
    edge_index = np.asarray(edge_index)
    nchunk, meta = prep_edges(cfg, edge_index)
    h1 = run_layer(
        cfg,
        nchunk,
        meta,
        x,
        np.asarray(W1, np.float32),
        np.asarray(a_src1, np.float32),
        np.asarray(a_dst1, np.float32),
        np.asarray(b1, np.float32),
        layer=1,
    )
    out = run_layer(
        cfg,
        nchunk,
        meta,
        h1,
        np.asarray(W2, np.float32),
        np.asarray(a_src2, np.float32),
        np.asarray(a_dst2, np.float32),
        np.asarray(b2, np.float32),
        layer=2,
    )
    return out
